# revision 18
# baseline (speedup 1.0000x reference)
"""CapsuleLayer kernel for 8x Trainium2 NeuronCores.

Reference computes h = x @ W[0]  ([32,512]@[512,16384] -> [32,256,64] f32)
followed by 3 "routing" rounds:
    c = softmax(h, axis=1); h = einsum('bid,bjd->bjd', c, h)
The einsum contracts i only over c, so it equals h * sum_i c[b,i,d] = h * 1
(softmax sums to one over the contracted axis) -- the routing loop is the
identity up to f32 rounding (~1e-7 relative). The kernel therefore computes
just the matmul, sharded over the 16384-wide output dim across 8 cores so
each core streams a distinct 4 MiB slice of W (memory-bound roofline).

Numerics: x and W are split on the host into fp16 hi/lo pairs (exact to
~2^-22 relative) and (xh+xl)@(wh+wl) is computed at full fp16 PE rate with
fp32 PSUM accumulation -- fp32-class accuracy with the same 4 bytes/element
of HBM traffic.

PE efficiency: the stationary operand is [xh | xl] (128x64), so one pass of
a w stream produces both the xh and xl partial products, and the wh / wl
streams run concurrently on the two independent 128x64 column tiles of the
PE array. The partial blocks [xh@wh | xl@wh | xh@wl | xl@wl] land on
disjoint 32-partition PSUM ranges; block 0 leaves in f32, blocks 1+2
(~2^-11 of the result) leave in bf16, block 3 (~2^-22) is dropped, and the
host sums the blocks.

Raw Bass (no TileContext) with a hand-rolled feed-forward pipeline: every
buffer is written exactly once, so the only semaphores are the natural
producer->consumer edges. W streams in column chunks; the final chunks are
small so the end-of-kernel receipt->compute->copy->writeback chain is short.
"""

import os

import numpy as np

B = 32          # batch
K = 512         # in_dim (contraction)
N_FULL = 16384  # num_capsules * out_dim
NUM_CAPS = 256
OUT_DIM = 64
NUM_CORES = 8
N_SHARD = N_FULL // NUM_CORES  # 2048 columns per core

KI = 128            # contraction partition tile
KO = K // KI        # 4 contraction subtiles
# Column-chunk widths per core (sum = N_SHARD). Uniform while streaming,
# tapering at the end to shorten the kernel tail.
CHUNKS = [256] * 7 + [192, 64]
assert sum(CHUNKS) == N_SHARD
NCH = len(CHUNKS)
OFFS = [sum(CHUNKS[:i]) for i in range(NCH)]
N_BODY = 7                       # chunks with bf16 lo-block outputs
TAIL_W = sum(CHUNKS[N_BODY:])    # tail chunks leave f32 via one shared tile
TAIL_OFF = OFFS[N_BODY]
N_PSUM = 8          # PSUM banks; chunks beyond 8 reuse bank (j % N_PSUM)
N_WARM = int(os.environ.get("CAPS_WARM", "16"))  # PE warmup matmuls (HAM ramp)

_NC = None
LAST_RESULTS = None  # BassKernelResults of the most recent run (for profiling)


def _build_nc():
    import concourse.bass as bass
    import concourse.mybir as mybir

    f16 = mybir.dt.float16
    f32 = mybir.dt.float32
    bf16 = mybir.dt.bfloat16
    nc = bass.Bass("TRN2", target_bir_lowering=False)

    # Host-prepacked fp16 hi/lo pairs, contiguous per partition:
    #  xp[ki, ko, s, b]  = split(x)[s][b, ko*KI + ki]          (s = hi/lo)
    #  wp[ki, chunk-major: (s, ko, t)] = split(W)[s][ko*KI + ki, n0 + off_j + t]
    xp = nc.dram_tensor("xp", [KI, KO * 2 * B], f16, kind="ExternalInput")
    wp = nc.dram_tensor("wp", [KI, 2 * KO * N_SHARD], f16, kind="ExternalInput")
    out_hi = nc.dram_tensor("out_hi", [B, TAIL_OFF], f32, kind="ExternalOutput")
    out_lo = nc.dram_tensor("out_lo", [2 * B, TAIL_OFF], bf16, kind="ExternalOutput")
    # Tail chunks: single f32 [96, w] copy + one DMA keeps the kernel tail
    # short (no DVE triple-copy backlog at the end).
    out_tail = nc.dram_tensor("out_tail", [3 * B, TAIL_W], f32, kind="ExternalOutput")

    x_tile = nc.alloc_sbuf_tensor("x_tile", [KI, KO * 2 * B], f16)
    w_tiles = [
        nc.alloc_sbuf_tensor(f"w_tile{j}", [KI, 2 * KO * CHUNKS[j]], f16)
        for j in range(NCH)
    ]
    oh_tiles = [
        nc.alloc_sbuf_tensor(f"oh_tile{j}", [B, CHUNKS[j]], f32)
        for j in range(N_BODY)
    ]
    # DVE is lane-locked, so the lo tiles sit on partitions 32:96 to match
    # the PSUM blocks they copy from (rows 0:32 are unused padding).
    ol_tiles = [
        nc.alloc_sbuf_tensor(f"ol_tile{j}", [3 * B, CHUNKS[j]], bf16)
        for j in range(N_BODY)
    ]
    o_tail = nc.alloc_sbuf_tensor("o_tail", [3 * B, TAIL_W], f32)
    warm_tile = nc.alloc_sbuf_tensor("warm_tile", [KI, 256], f16)

    NT_MAX = max(CHUNKS)
    ps_tiles = [
        nc.alloc_psum_tensor(f"ps{p}", [4 * B, NT_MAX], f32) for p in range(N_PSUM)
    ]
    ps_warm = ps_tiles[0]  # warmup matmuls run before chunk 0; start=True resets

    x_ap = x_tile.ap().rearrange("ki (ko sb) -> ki ko sb", ko=KO)  # sb = 64
    w_aps = [
        w.ap().rearrange("ki (s ko t) -> ki s ko t", s=2, ko=KO) for w in w_tiles
    ]

    x_sem = nc.alloc_semaphore("x_sem")
    # One sem per W chunk: a shared counter is racy because each DMA's 16
    # SDMA engines increment independently and can skew across chunks.
    w_sems = [nc.alloc_semaphore(f"w_sem{j}") for j in range(NCH)]
    warm_sem = nc.alloc_semaphore("warm_sem")
    mm_sem = nc.alloc_semaphore("mm_sem")
    cph_sem = nc.alloc_semaphore("cph_sem")
    cpl_sem = nc.alloc_semaphore("cpl_sem")
    outh_sem = nc.alloc_semaphore("outh_sem")
    outl_sem = nc.alloc_semaphore("outl_sem")

    with nc.Block() as block:

        @block.gpsimd
        def _(gpsimd):
            gpsimd.memset(warm_tile[:], 0).then_inc(warm_sem, 1)
            gpsimd.dma_start(x_tile[:], xp[:]).then_inc(x_sem, 16)

        @block.sync
        def _(sync):
            for j in range(NCH):
                sync.dma_start(
                    w_tiles[j][:],
                    wp[:, 2 * KO * OFFS[j] : 2 * KO * (OFFS[j] + CHUNKS[j])],
                ).then_inc(w_sems[j], 16)
            # Sync is free once the W loads are queued; it ships the bf16
            # lo blocks while Scalar ships the f32 hi blocks in parallel.
            for j in range(N_BODY):
                sync.wait_ge(cpl_sem, j + 1)
                sync.dma_start(
                    out_lo[:, OFFS[j] : OFFS[j] + CHUNKS[j]],
                    ol_tiles[j].ap()[B : 3 * B],
                ).then_inc(outl_sem, 16)
            sync.wait_ge(outl_sem, 16 * N_BODY)

        @block.tensor
        def _(tensor):
            tensor.wait_ge(warm_sem, 1)
            for i in range(N_WARM):
                half = (i % 2) * 2 * B
                tensor.matmul(
                    ps_warm.ap()[half : half + 2 * B, :256],
                    warm_tile[:, : 2 * B],
                    warm_tile[:],
                    start=True,
                    stop=True,
                )
            tensor.wait_ge(x_sem, 16)
            for j in range(NCH):
                tensor.wait_ge(w_sems[j], 16)
                if j >= N_PSUM:
                    # Bank reuse: the copies of chunk j - N_PSUM must be done.
                    tensor.wait_ge(cph_sem, j - N_PSUM + 1)
                    tensor.wait_ge(cpl_sem, j - N_PSUM + 1)
                ps = ps_tiles[j % N_PSUM]
                # Column tile s (s=0: psum rows 0:64, s=1: rows 64:128) runs
                # the w_s stream; ko accumulates within each tile. The tiles
                # complete independently, so the copy waits on BOTH tiles'
                # final matmuls (2 incs per chunk).
                for ko in range(KO):
                    for s in range(2):
                        ins = tensor.matmul(
                            ps.ap()[s * 2 * B : (s + 1) * 2 * B, : CHUNKS[j]],
                            x_ap[:, ko, :],
                            w_aps[j][:, s, ko, :],
                            start=(ko == 0),
                            stop=(ko == KO - 1),
                        )
                        if ko == KO - 1:
                            ins.then_inc(mm_sem, 1)

        @block.vector
        def _(vector):
            for j in range(N_BODY):
                vector.wait_ge(mm_sem, 2 * (j + 1))
                ps = ps_tiles[j % N_PSUM]
                vector.tensor_copy(
                    oh_tiles[j][:], ps.ap()[:B, : CHUNKS[j]]
                ).then_inc(cph_sem, 1)
                # PSUM access patterns may span at most 32 partitions when
                # starting at partition 32 -> two lane-aligned copies.
                vector.tensor_copy(
                    ol_tiles[j].ap()[B : 2 * B], ps.ap()[B : 2 * B, : CHUNKS[j]]
                )
                vector.tensor_copy(
                    ol_tiles[j].ap()[2 * B : 3 * B],
                    ps.ap()[2 * B : 3 * B, : CHUNKS[j]],
                ).then_inc(cpl_sem, 1)
            for j in range(N_BODY, NCH):
                vector.wait_ge(mm_sem, 2 * (j + 1))
                ps = ps_tiles[j % N_PSUM]
                off = OFFS[j] - TAIL_OFF
                vector.tensor_copy(
                    o_tail.ap()[:, off : off + CHUNKS[j]],
                    ps.ap()[: 3 * B, : CHUNKS[j]],
                ).then_inc(cph_sem, 1)

        @block.scalar
        def _(scalar):
            for j in range(N_BODY):
                scalar.wait_ge(cph_sem, j + 1)
                scalar.dma_start(
                    out_hi[:, OFFS[j] : OFFS[j] + CHUNKS[j]], oh_tiles[j][:]
                ).then_inc(outh_sem, 16)
            scalar.wait_ge(cph_sem, NCH)
            scalar.dma_start(out_tail[:], o_tail[:]).then_inc(outh_sem, 16)
            scalar.wait_ge(outh_sem, 16 * (N_BODY + 1))

    return nc


def _get_nc():
    global _NC
    if _NC is None:
        _NC = _build_nc()
    return _NC


def _split_f16(a):
    hi = a.astype(np.float16)
    lo = (a - hi.astype(np.float32)).astype(np.float16)
    return hi, lo


def kernel(x, W):
    global LAST_RESULTS
    from concourse.bass_utils import run_bass_kernel_spmd

    x = np.ascontiguousarray(np.asarray(x, dtype=np.float32))
    W2 = np.ascontiguousarray(np.asarray(W, dtype=np.float32)).reshape(K, N_FULL)

    xh, xl = _split_f16(x)
    wh, wl = _split_f16(W2)

    # xp[ki, ko, s, b] = x_s[b, ko*KI + ki]  -> [KI, KO*2*B]
    xs = np.stack([xh, xl])  # [2, B, K]
    xp = np.ascontiguousarray(
        xs.transpose(2, 0, 1).reshape(KO, KI, 2, B).transpose(1, 0, 2, 3).reshape(
            KI, KO * 2 * B
        )
    )
    # wk[ki, s, ko, n] = w_s[ko*KI + ki, n]  (full width, then chunk-sliced)
    ws = np.stack([wh, wl])  # [2, K, N]
    wk = ws.reshape(2, KO, KI, N_FULL).transpose(2, 0, 1, 3)  # [KI, 2, KO, N]

    nc = _get_nc()
    in_maps = []
    for c in range(NUM_CORES):
        n0 = c * N_SHARD
        # Chunk-major packing: per partition, chunk j's (s, ko, t) block is
        # contiguous so each chunk is a single contiguous-per-partition DMA.
        blocks = [
            wk[:, :, :, n0 + OFFS[j] : n0 + OFFS[j] + CHUNKS[j]].reshape(KI, -1)
            for j in range(NCH)
        ]
        wp = np.ascontiguousarray(np.concatenate(blocks, axis=1))
        in_maps.append({"xp": xp, "wp": wp})

    res = run_bass_kernel_spmd(nc, in_maps, core_ids=list(range(NUM_CORES)))
    LAST_RESULTS = res
    # out = hi block + the two cross-term blocks, stitched across cores.
    parts = []
    for r in res.results:
        body = r["out_hi"] + r["out_lo"].astype(np.float32).reshape(
            2, B, TAIL_OFF
        ).sum(axis=0)
        tail = r["out_tail"].reshape(3, B, TAIL_W).sum(axis=0)
        parts.append(np.concatenate([body, tail], axis=1))
    full = np.concatenate(parts, axis=1)
    return full.reshape(B, NUM_CAPS, OUT_DIM).astype(np.float32)
